# revision 5
# baseline (speedup 1.0000x reference)
"""FALCON ObjectSomeValuesFrom forward kernel for Trainium2 (Bass/Tile).

Math: the reference computes
    c_fs[j]   = sigmoid(cw + col_j + b)
    r_fs[i,j] = sigmoid(row_i + col_j + b)
    out[i]    = max_j r_fs[i,j] * c_fs[j]
with col_j = e_j . w_r, row_i = e_i . w_l + rw, cw = c_emb . w_l,
rw = r_emb . w_l.  Both product factors are strictly increasing in col_j,
so the max over j is attained at argmax_j col_j for every i:
    out[i] = sigmoid(a_i + rw + colmax + b) * sigmoid(cw + colmax + b)
with a_i = e_i . w_l and colmax = max_j col_j.  The O(N^2) pairwise block
collapses to two GEMVs over e_all plus an elementwise sigmoid tail.

Sharding: the entity axis is split 8 ways (1024 rows per core).  Each
core loads ONLY its own shard — [128, 2+1024] fp8-e3m4 (w_r, w_l in the
first two columns, the transposed shard after) — computes both GEMVs as
8 PE matmuls ([K=128, M=128] stationary x [K=128, N=2] moving, PSUM
[128, 8, 2]), and a DVE free-axis reduce_max over its 8 col-dot columns
(the shard-local max-reduction over j).  The raw PSUM block [128, 17]
(8x2 dots + the 128 partial maxima) is DMA'd out in f32.  The host-side
gather/unshard combines the 8 shards: colmax = max over the 8x128
partial maxima (8 scalars of real reduction work), then the elementwise
sigmoid finish over the gathered a-vector.  No cross-device
communication — the cross-core max rides the output gather, per the
sharding plan.

Why sharded: every-core-scans-the-full-table (the previous design) is
bound by the serialized 1 MB HBM read — 2.9 us of DMA on top of a
~6.1 us fixed-latency floor (DMA issue 650 + HWDGE 625 + DGE 650 +
completion semaphore 900 per direction, plus pre/postamble barriers).
An 8x smaller shard read (365 ns) puts the kernel at that floor, and a
cross-core collective instead would cost ~28 us (AllReduce constant
overhead), far more than it saves.

Critical path per core: preamble -> one input DMA (SP/HWDGE) -> 8
matmul pairs -> one DVE reduce -> one output DMA.  The two GEMV dots
land interleaved in PSUM ([p, 2c] = col-dot, [p, 2c+1] = a-dot of shard
entity c*128+p) and the DVE max lands in column 16, so a single
contiguous [128, 17] f32 DMA ships everything (descriptor floor,
~56 ns) with no SBUF copy, no activation-table load, and no Pool
partition-reduce on the critical path.
"""

import numpy as np

N = 8192        # 8000 named + 192 anon entities
D = 128         # emb dim == contraction == partitions
P = 128
NCORES = 8
RPC = N // NCORES     # rows per core (1024)
OWN = RPC // P        # 8 chunks of 128 rows per core
OUTC = 2 * OWN + 1    # 16 dot columns + 1 max column
COL_DT = "fp8e3"      # "fp8e4" | "fp8e3" | "fp16" | "bf16"
SCALE = {"fp8e4": 8.0, "fp8e3": 32.0, "fp16": 1.0, "bf16": 1.0}

_CACHE = {}


def _np_dt(col_dt):
    import ml_dtypes
    return {
        "fp8e4": ml_dtypes.float8_e4m3,
        "fp8e3": ml_dtypes.float8_e3m4,
        "fp16": np.float16,
        "bf16": ml_dtypes.bfloat16,
    }[col_dt]


def _build_nc(repeat=1, col_dt=COL_DT):
    import concourse.bass as bass  # noqa: F401  (env preload)
    import concourse.bacc as bacc
    import concourse.tile as tile
    import concourse.mybir as mybir

    f32 = mybir.dt.float32
    cdt = {
        "fp8e4": mybir.dt.float8e4,
        "fp8e3": mybir.dt.float8e3,
        "fp16": mybir.dt.float16,
        "bf16": mybir.dt.bfloat16,
    }[col_dt]
    nc = bacc.Bacc("TRN2", target_bir_lowering=False, debug=False)

    # Cols 0:2 = [w_r, w_l] (scaled); 2:RPC+2 = this core's shard of the
    # scaled/transposed e-table.  One DMA covers every input.
    et_d = nc.dram_tensor("et", [P, RPC + 2], cdt, kind="ExternalInput").ap()
    out_d = nc.dram_tensor("out", [P, OUTC], f32, kind="ExternalOutput").ap()

    with tile.TileContext(nc) as tc:
        with (
            tc.tile_pool(name="sb", bufs=1) as sb,
            tc.tile_pool(name="ps", bufs=1, space="PSUM") as ps,
        ):
            et = sb.tile([P, RPC + 2], cdt)
            nc.sync.dma_start(et[:], et_d[:])

            # Dependency-free dummy copy: if the compiler decides the
            # Activation engine needs an act-table load for Copy, it lands
            # here, inside the DMA window, not on the critical tail.
            dum = sb.tile([P, 1], f32)
            nc.vector.memset(dum[:], 0.0)
            dum2 = sb.tile([P, 1], f32)
            nc.scalar.copy(dum2[:], dum[:])

            w2 = et[:, 0:2]
            pst = ps.tile([P, 2 * OWN], f32)
            psv = pst[:].rearrange("p (n two) -> p n two", two=2)
            for _ in range(repeat):
                for c in range(OWN):
                    nc.tensor.matmul(
                        psv[:, c, :],
                        et[:, 2 + c * P : 2 + (c + 1) * P],
                        w2,
                        start=True,
                        stop=True,
                    )

            # PSUM cannot DMA to DRAM directly: stage through SBUF.  The
            # dots copy (Activation engine) runs in parallel with the
            # shard-local max over j (DVE, per-partition max of the 8
            # col-dot columns); they land adjacently so one DMA ships both.
            ot = sb.tile([P, OUTC], f32)
            nc.scalar.copy(ot[:, 0 : 2 * OWN], pst[:])
            nc.vector.reduce_max(
                ot[:, 2 * OWN : OUTC], psv[:, :, 0], axis=mybir.AxisListType.X
            )

            nc.sync.dma_start(out_d[:], ot[:])

    nc.compile()
    return nc


def get_nc(repeat=1, col_dt=COL_DT):
    key = ("nc", repeat, col_dt)
    if key not in _CACHE:
        _CACHE[key] = _build_nc(repeat, col_dt)
    return _CACHE[key]


def prep(anon_e_emb, e_table, c_table, r_table, fc0_w, fc0_b, c_id, r_id,
         col_dt=COL_DT):
    """Host prep: shard + quantize inputs; return (in_maps, aux for finish)."""
    e_all = np.concatenate(
        [np.asarray(e_table, np.float32), np.asarray(anon_e_emb, np.float32)], 0
    )  # [N, D]
    fc0_w = np.asarray(fc0_w, np.float32)
    w_l = fc0_w[0, :D]
    w_r = fc0_w[0, D:]
    b = np.float32(np.asarray(fc0_b, np.float32)[0])
    c_emb = np.asarray(c_table, np.float32)[int(c_id)]
    r_emb = np.asarray(r_table, np.float32)[int(r_id)]
    rw = np.float32(np.dot(r_emb, w_l))
    cw = np.float32(np.dot(c_emb, w_l))

    s = SCALE[col_dt]
    ndt = _np_dt(col_dt)
    eT = np.ascontiguousarray((e_all.T * s).astype(ndt))  # [D, N] quantized
    wq = np.stack([(w_r * s).astype(ndt), (w_l * s).astype(ndt)], axis=1)

    in_maps = []
    for core in range(NCORES):
        aug = np.empty((P, RPC + 2), ndt)
        aug[:, 0:2] = wq
        aug[:, 2:] = eT[:, core * RPC : (core + 1) * RPC]
        in_maps.append({"et": np.ascontiguousarray(aug)})

    aux = {"rw": rw, "cw": cw, "b": b, "inv_s2": np.float32(1.0 / (s * s))}
    return in_maps, aux


def host_finish(core_outs, aux):
    """Gather/unshard: 8-scalar max across shards + elementwise finish.

    core_outs[c] is core c's [128, 17] f32 block: [p, 2c+1] = a-dot of
    shard entity c*128+p (x S^2), [p, 16] = shard-local col max (x S^2).
    """
    inv_s2 = aux["inv_s2"]
    colmax = max(o[:, 2 * OWN].max() for o in core_outs) * inv_s2
    a = np.concatenate(
        [o[:, 1 : 2 * OWN : 2].T.reshape(-1) for o in core_outs]
    ) * inv_s2

    def sigmoid(x):
        return 1.0 / (1.0 + np.exp(-x))

    k1 = a + (aux["rw"] + aux["b"] + colmax)
    k2 = sigmoid(np.float32(aux["cw"] + aux["b"]) + colmax)
    return (sigmoid(k1) * k2).astype(np.float32)


def run(inputs, trace=False, trace_kwargs=None, repeat=1, col_dt=COL_DT):
    from concourse.bass_utils import run_bass_kernel_spmd

    nc = get_nc(repeat, col_dt)
    in_maps, aux = prep(**inputs, col_dt=col_dt)
    res = run_bass_kernel_spmd(
        nc,
        in_maps,
        core_ids=list(range(NCORES)),
        trace=trace,
        **(trace_kwargs or {}),
    )
    out = host_finish(
        [np.asarray(res.results[c]["out"], np.float32) for c in range(NCORES)],
        aux,
    )
    return out, res


def kernel(**inputs) -> np.ndarray:
    out, _ = run(inputs, trace=False)
    return out


# revision 8
# speedup vs baseline: 1.0479x; 1.0479x over previous
"""FALCON ObjectSomeValuesFrom forward kernel for Trainium2 (Bass/Tile).

Math: the reference computes
    c_fs[j]   = sigmoid(cw + col_j + b)
    r_fs[i,j] = sigmoid(row_i + col_j + b)
    out[i]    = max_j r_fs[i,j] * c_fs[j]
with col_j = e_j . w_r, row_i = e_i . w_l + rw, cw = c_emb . w_l,
rw = r_emb . w_l.  Both product factors are strictly increasing in col_j,
so the max over j is attained at argmax_j col_j for every i:
    out[i] = sigmoid(a_i + rw + colmax + b) * sigmoid(cw + colmax + b)
with a_i = e_i . w_l and colmax = max_j col_j.  The O(N^2) pairwise block
collapses to two GEMVs over e_all plus an elementwise sigmoid tail.

Sharding: the entity axis is split 8 ways (1024 rows per core).  Each
core loads ONLY its own shard — [128, 2+1024] fp8-e3m4 (w_r, w_l in the
first two columns, the transposed shard after) — computes both GEMVs as
8 PE matmuls ([K=128, M=128] stationary x [K=128, N=2] moving, PSUM
[128, 8, 2]), and a DVE free-axis reduce_max over its 8 col-dot columns
(the shard-local max-reduction over j).  The raw PSUM block [128, 17]
(8x2 dots + the 128 partial maxima) is DMA'd out in f32.  The host-side
gather/unshard combines the 8 shards: colmax = max over the 8x128
partial maxima (8 scalars of real reduction work), then the elementwise
sigmoid finish over the gathered a-vector.  No cross-device
communication — the cross-core max rides the output gather, per the
sharding plan.

Why sharded: every-core-scans-the-full-table (the previous design) is
bound by the serialized 1 MB HBM read — 2.9 us of DMA on top of a
~6.1 us fixed-latency floor (DMA issue 650 + HWDGE 625 + DGE 650 +
completion semaphore 900 per direction, plus pre/postamble barriers).
An 8x smaller shard read (365 ns) puts the kernel at that floor, and a
cross-core collective instead would cost ~28 us (AllReduce constant
overhead), far more than it saves.

Critical path per core: preamble -> one input DMA (SP/HWDGE) -> 8
matmul pairs -> one DVE reduce -> one output DMA.  The two GEMV dots
land interleaved in PSUM ([p, 2c] = col-dot, [p, 2c+1] = a-dot of shard
entity c*128+p) and the DVE max lands in column 16, so a single
contiguous [128, 17] f32 DMA ships everything (descriptor floor,
~56 ns) with no SBUF copy, no activation-table load, and no Pool
partition-reduce on the critical path.
"""

import numpy as np

N = 8192        # 8000 named + 192 anon entities
D = 128         # emb dim == contraction == partitions
P = 128
NCORES = 8
RPC = N // NCORES     # rows per core (1024)
OWN = RPC // P        # 8 chunks of 128 rows per core
OUTC = OWN + 1        # 8 a-dot columns + 1 max column
COL_DT = "fp8e3"      # "fp8e4" | "fp8e3" | "fp16" | "bf16"
SCALE = {"fp8e4": 8.0, "fp8e3": 32.0, "fp16": 1.0, "bf16": 1.0}

_CACHE = {}


def _np_dt(col_dt):
    import ml_dtypes
    return {
        "fp8e4": ml_dtypes.float8_e4m3,
        "fp8e3": ml_dtypes.float8_e3m4,
        "fp16": np.float16,
        "bf16": ml_dtypes.bfloat16,
    }[col_dt]


def _build_nc(repeat=1, col_dt=COL_DT):
    import concourse.bass as bass  # noqa: F401  (env preload)
    import concourse.bacc as bacc
    import concourse.tile as tile
    import concourse.mybir as mybir

    f32 = mybir.dt.float32
    cdt = {
        "fp8e4": mybir.dt.float8e4,
        "fp8e3": mybir.dt.float8e3,
        "fp16": mybir.dt.float16,
        "bf16": mybir.dt.bfloat16,
    }[col_dt]
    nc = bacc.Bacc("TRN2", target_bir_lowering=False, debug=False)

    # Cols 0:2 = [w_r, w_l] (scaled); 2:RPC+2 = this core's shard of the
    # scaled/transposed e-table.  One DMA covers every input.
    et_d = nc.dram_tensor("et", [P, RPC + 2], cdt, kind="ExternalInput").ap()
    out_d = nc.dram_tensor("out", [P, OUTC], f32, kind="ExternalOutput").ap()

    with tile.TileContext(nc) as tc:
        with (
            tc.tile_pool(name="sb", bufs=1) as sb,
            tc.tile_pool(name="ps", bufs=1, space="PSUM") as ps,
        ):
            et = sb.tile([P, RPC + 2], cdt)
            nc.sync.dma_start(et[:], et_d[:])

            # Dependency-free dummy copy: if the compiler decides the
            # Activation engine needs an act-table load for Copy, it lands
            # here, inside the DMA window, not on the critical tail.
            dum = sb.tile([P, 1], f32)
            nc.vector.memset(dum[:], 0.0)
            dum2 = sb.tile([P, 1], f32)
            nc.scalar.copy(dum2[:], dum[:])

            # Two PSUM tiles, one per GEMV: the tail's two readers (Act
            # copy of psA, DVE reduce of psB) would be falsely serialized
            # by tile's cross-engine same-PSUM-tile read hazard otherwise.
            w_r = et[:, 0:1]
            w_l = et[:, 1:2]
            psA = ps.tile([P, OWN], f32)   # a-dots  (w_l)
            psB = ps.tile([P, OWN], f32)   # col-dots (w_r)
            for _ in range(repeat):
                for c in range(OWN):
                    chunk = et[:, 2 + c * P : 2 + (c + 1) * P]
                    nc.tensor.matmul(
                        psA[:, c : c + 1], chunk, w_l, start=True, stop=True
                    )
                    nc.tensor.matmul(
                        psB[:, c : c + 1], chunk, w_r, start=True, stop=True
                    )

            # PSUM cannot DMA to DRAM directly: stage through SBUF.  The
            # a-dots copy (Activation engine) runs in parallel with the
            # shard-local max over j (DVE, per-partition max of the 8
            # col-dot columns); they land adjacently so one DMA ships both.
            ot = sb.tile([P, OUTC], f32)
            nc.scalar.copy(ot[:, 0:OWN], psA[:])
            nc.vector.reduce_max(
                ot[:, OWN:OUTC], psB[:], axis=mybir.AxisListType.X
            )

            nc.sync.dma_start(out_d[:], ot[:])

    nc.compile()
    return nc


def get_nc(repeat=1, col_dt=COL_DT):
    key = ("nc", repeat, col_dt)
    if key not in _CACHE:
        _CACHE[key] = _build_nc(repeat, col_dt)
    return _CACHE[key]


def prep(anon_e_emb, e_table, c_table, r_table, fc0_w, fc0_b, c_id, r_id,
         col_dt=COL_DT):
    """Host prep: shard + quantize inputs; return (in_maps, aux for finish)."""
    e_all = np.concatenate(
        [np.asarray(e_table, np.float32), np.asarray(anon_e_emb, np.float32)], 0
    )  # [N, D]
    fc0_w = np.asarray(fc0_w, np.float32)
    w_l = fc0_w[0, :D]
    w_r = fc0_w[0, D:]
    b = np.float32(np.asarray(fc0_b, np.float32)[0])
    c_emb = np.asarray(c_table, np.float32)[int(c_id)]
    r_emb = np.asarray(r_table, np.float32)[int(r_id)]
    rw = np.float32(np.dot(r_emb, w_l))
    cw = np.float32(np.dot(c_emb, w_l))

    s = SCALE[col_dt]
    ndt = _np_dt(col_dt)
    eT = np.ascontiguousarray((e_all.T * s).astype(ndt))  # [D, N] quantized
    wq = np.stack([(w_r * s).astype(ndt), (w_l * s).astype(ndt)], axis=1)

    in_maps = []
    for core in range(NCORES):
        aug = np.empty((P, RPC + 2), ndt)
        aug[:, 0:2] = wq
        aug[:, 2:] = eT[:, core * RPC : (core + 1) * RPC]
        in_maps.append({"et": np.ascontiguousarray(aug)})

    aux = {"rw": rw, "cw": cw, "b": b, "inv_s2": np.float32(1.0 / (s * s))}
    return in_maps, aux


def host_finish(core_outs, aux):
    """Gather/unshard: 8-scalar max across shards + elementwise finish.

    core_outs[c] is core c's [128, 9] f32 block: [p, c] = a-dot of
    shard entity c*128+p (x S^2), [p, 8] = shard-local col max (x S^2).
    """
    inv_s2 = aux["inv_s2"]
    colmax = max(o[:, OWN].max() for o in core_outs) * inv_s2
    a = np.concatenate(
        [o[:, 0:OWN].T.reshape(-1) for o in core_outs]
    ) * inv_s2

    def sigmoid(x):
        return 1.0 / (1.0 + np.exp(-x))

    k1 = a + (aux["rw"] + aux["b"] + colmax)
    k2 = sigmoid(np.float32(aux["cw"] + aux["b"]) + colmax)
    return (sigmoid(k1) * k2).astype(np.float32)


def run(inputs, trace=False, trace_kwargs=None, repeat=1, col_dt=COL_DT):
    from concourse.bass_utils import run_bass_kernel_spmd

    nc = get_nc(repeat, col_dt)
    in_maps, aux = prep(**inputs, col_dt=col_dt)
    res = run_bass_kernel_spmd(
        nc,
        in_maps,
        core_ids=list(range(NCORES)),
        trace=trace,
        **(trace_kwargs or {}),
    )
    out = host_finish(
        [np.asarray(res.results[c]["out"], np.float32) for c in range(NCORES)],
        aux,
    )
    return out, res


def kernel(**inputs) -> np.ndarray:
    out, _ = run(inputs, trace=False)
    return out


# revision 9
# speedup vs baseline: 1.0523x; 1.0042x over previous
"""FALCON ObjectSomeValuesFrom forward kernel for Trainium2 (Bass/Tile).

Math: the reference computes
    c_fs[j]   = sigmoid(cw + col_j + b)
    r_fs[i,j] = sigmoid(row_i + col_j + b)
    out[i]    = max_j r_fs[i,j] * c_fs[j]
with col_j = e_j . w_r, row_i = e_i . w_l + rw, cw = c_emb . w_l,
rw = r_emb . w_l.  Both product factors are strictly increasing in col_j,
so the max over j is attained at argmax_j col_j for every i:
    out[i] = sigmoid(a_i + rw + colmax + b) * sigmoid(cw + colmax + b)
with a_i = e_i . w_l and colmax = max_j col_j.  The O(N^2) pairwise block
collapses to two GEMVs over e_all plus an elementwise sigmoid tail.

Sharding: the entity axis is split 8 ways (1024 rows per core).  Each
core loads ONLY its own shard — [128, 2+1024] fp8-e3m4 (w_r, w_l in the
first two columns, the transposed shard after) — computes both GEMVs as
8 PE matmuls ([K=128, M=128] stationary x [K=128, N=2] moving, PSUM
[128, 8, 2]), and a DVE free-axis reduce_max over its 8 col-dot columns
(the shard-local max-reduction over j).  The raw PSUM block [128, 17]
(8x2 dots + the 128 partial maxima) is DMA'd out in f32.  The host-side
gather/unshard combines the 8 shards: colmax = max over the 8x128
partial maxima (8 scalars of real reduction work), then the elementwise
sigmoid finish over the gathered a-vector.  No cross-device
communication — the cross-core max rides the output gather, per the
sharding plan.

Why sharded: every-core-scans-the-full-table (the previous design) is
bound by the serialized 1 MB HBM read — 2.9 us of DMA on top of a
~6.1 us fixed-latency floor (DMA issue 650 + HWDGE 625 + DGE 650 +
completion semaphore 900 per direction, plus pre/postamble barriers).
An 8x smaller shard read (365 ns) puts the kernel at that floor, and a
cross-core collective instead would cost ~28 us (AllReduce constant
overhead), far more than it saves.

Critical path per core: preamble -> one input DMA (SP/HWDGE) -> 8
matmul pairs -> one DVE reduce -> one output DMA.  The two GEMV dots
land interleaved in PSUM ([p, 2c] = col-dot, [p, 2c+1] = a-dot of shard
entity c*128+p) and the DVE max lands in column 16, so a single
contiguous [128, 17] f32 DMA ships everything (descriptor floor,
~56 ns) with no SBUF copy, no activation-table load, and no Pool
partition-reduce on the critical path.
"""

import numpy as np

N = 8192        # 8000 named + 192 anon entities
D = 128         # emb dim == contraction == partitions
P = 128
NCORES = 8
RPC = N // NCORES     # rows per core (1024)
OWN = RPC // P        # 8 chunks of 128 rows per core
OUTC = OWN + 1        # 8 a-dot columns + 1 max column
COL_DT = "fp8e3"      # "fp8e4" | "fp8e3" | "fp16" | "bf16"
SCALE = {"fp8e4": 8.0, "fp8e3": 32.0, "fp16": 1.0, "bf16": 1.0}

_CACHE = {}


def _np_dt(col_dt):
    import ml_dtypes
    return {
        "fp8e4": ml_dtypes.float8_e4m3,
        "fp8e3": ml_dtypes.float8_e3m4,
        "fp16": np.float16,
        "bf16": ml_dtypes.bfloat16,
    }[col_dt]


def _build_nc(repeat=1, col_dt=COL_DT):
    import concourse.bass as bass  # noqa: F401  (env preload)
    import concourse.bacc as bacc
    import concourse.tile as tile
    import concourse.mybir as mybir

    f32 = mybir.dt.float32
    cdt = {
        "fp8e4": mybir.dt.float8e4,
        "fp8e3": mybir.dt.float8e3,
        "fp16": mybir.dt.float16,
        "bf16": mybir.dt.bfloat16,
    }[col_dt]
    nc = bacc.Bacc("TRN2", target_bir_lowering=False, debug=False)

    # Cols 0:2 = [w_r, w_l] (scaled); 2:RPC+2 = this core's shard of the
    # scaled/transposed e-table.  One DMA covers every input.
    et_d = nc.dram_tensor("et", [P, RPC + 2], cdt, kind="ExternalInput").ap()
    out_d = nc.dram_tensor("out", [P, OUTC], f32, kind="ExternalOutput").ap()

    with tile.TileContext(nc) as tc:
        with (
            tc.tile_pool(name="sb", bufs=1) as sb,
            tc.tile_pool(name="ps", bufs=1, space="PSUM") as ps,
        ):
            et = sb.tile([P, RPC + 2], cdt)
            nc.sync.dma_start(et[:], et_d[:])

            # Dependency-free dummy copy: if the compiler decides the
            # Activation engine needs an act-table load for Copy, it lands
            # here, inside the DMA window, not on the critical tail.
            dum = sb.tile([P, 1], f32)
            nc.vector.memset(dum[:], 0.0)
            dum2 = sb.tile([P, 1], f32)
            nc.scalar.copy(dum2[:], dum[:])

            # Two PSUM tiles, one per GEMV: the tail's two readers (Act
            # copy of psA, DVE reduce of psB) would be falsely serialized
            # by tile's cross-engine same-PSUM-tile read hazard otherwise.
            w_r = et[:, 0:1]
            w_l = et[:, 1:2]
            psA = ps.tile([P, OWN], f32)   # a-dots  (w_l)
            psB = ps.tile([P, OWN], f32)   # col-dots (w_r)
            # psA matmuls first: the Act copy (the longer tail op) waits
            # only on these 8, releasing ~30ns earlier than interleaved.
            for _ in range(repeat):
                for c in range(OWN):
                    chunk = et[:, 2 + c * P : 2 + (c + 1) * P]
                    nc.tensor.matmul(
                        psA[:, c : c + 1], chunk, w_l, start=True, stop=True
                    )
                for c in range(OWN):
                    chunk = et[:, 2 + c * P : 2 + (c + 1) * P]
                    nc.tensor.matmul(
                        psB[:, c : c + 1], chunk, w_r, start=True, stop=True
                    )

            # PSUM cannot DMA to DRAM directly: stage through SBUF.  The
            # a-dots copy (Activation engine) runs in parallel with the
            # shard-local max over j (DVE, per-partition max of the 8
            # col-dot columns); they land adjacently so one DMA ships both.
            ot = sb.tile([P, OUTC], f32)
            nc.scalar.copy(ot[:, 0:OWN], psA[:])
            nc.vector.reduce_max(
                ot[:, OWN:OUTC], psB[:], axis=mybir.AxisListType.X
            )

            nc.sync.dma_start(out_d[:], ot[:])

    nc.compile()
    return nc


def get_nc(repeat=1, col_dt=COL_DT):
    key = ("nc", repeat, col_dt)
    if key not in _CACHE:
        _CACHE[key] = _build_nc(repeat, col_dt)
    return _CACHE[key]


def prep(anon_e_emb, e_table, c_table, r_table, fc0_w, fc0_b, c_id, r_id,
         col_dt=COL_DT):
    """Host prep: shard + quantize inputs; return (in_maps, aux for finish)."""
    e_all = np.concatenate(
        [np.asarray(e_table, np.float32), np.asarray(anon_e_emb, np.float32)], 0
    )  # [N, D]
    fc0_w = np.asarray(fc0_w, np.float32)
    w_l = fc0_w[0, :D]
    w_r = fc0_w[0, D:]
    b = np.float32(np.asarray(fc0_b, np.float32)[0])
    c_emb = np.asarray(c_table, np.float32)[int(c_id)]
    r_emb = np.asarray(r_table, np.float32)[int(r_id)]
    rw = np.float32(np.dot(r_emb, w_l))
    cw = np.float32(np.dot(c_emb, w_l))

    s = SCALE[col_dt]
    ndt = _np_dt(col_dt)
    eT = np.ascontiguousarray((e_all.T * s).astype(ndt))  # [D, N] quantized
    wq = np.stack([(w_r * s).astype(ndt), (w_l * s).astype(ndt)], axis=1)

    in_maps = []
    for core in range(NCORES):
        aug = np.empty((P, RPC + 2), ndt)
        aug[:, 0:2] = wq
        aug[:, 2:] = eT[:, core * RPC : (core + 1) * RPC]
        in_maps.append({"et": np.ascontiguousarray(aug)})

    aux = {"rw": rw, "cw": cw, "b": b, "inv_s2": np.float32(1.0 / (s * s))}
    return in_maps, aux


def host_finish(core_outs, aux):
    """Gather/unshard: 8-scalar max across shards + elementwise finish.

    core_outs[c] is core c's [128, 9] f32 block: [p, c] = a-dot of
    shard entity c*128+p (x S^2), [p, 8] = shard-local col max (x S^2).
    """
    inv_s2 = aux["inv_s2"]
    colmax = max(o[:, OWN].max() for o in core_outs) * inv_s2
    a = np.concatenate(
        [o[:, 0:OWN].T.reshape(-1) for o in core_outs]
    ) * inv_s2

    def sigmoid(x):
        return 1.0 / (1.0 + np.exp(-x))

    k1 = a + (aux["rw"] + aux["b"] + colmax)
    k2 = sigmoid(np.float32(aux["cw"] + aux["b"]) + colmax)
    return (sigmoid(k1) * k2).astype(np.float32)


def run(inputs, trace=False, trace_kwargs=None, repeat=1, col_dt=COL_DT):
    from concourse.bass_utils import run_bass_kernel_spmd

    nc = get_nc(repeat, col_dt)
    in_maps, aux = prep(**inputs, col_dt=col_dt)
    res = run_bass_kernel_spmd(
        nc,
        in_maps,
        core_ids=list(range(NCORES)),
        trace=trace,
        **(trace_kwargs or {}),
    )
    out = host_finish(
        [np.asarray(res.results[c]["out"], np.float32) for c in range(NCORES)],
        aux,
    )
    return out, res


def kernel(**inputs) -> np.ndarray:
    out, _ = run(inputs, trace=False)
    return out
